# revision 15
# baseline (speedup 1.0000x reference)
"""GCN embedder kernel for TRN2, 8-core SPMD (v5: bf16, 4 SWDGE queues,
512-wide quad one-hots, DVE+Scalar split one-hot builds).

Design
------
* Nodes sharded contiguously across C=8 cores (NC nodes each). Edges
  (incl. self-loops) are owned by the dst core.
* Node features h are kept feature-major in SBUF as bf16: hT [H=128, NCP].
* Gather tables (T1 = emb@W1 for layer 1, hw_full = h@W_l for layers 2/3)
  are bf16 in DRAM; dma_gather cost is purely per-index, so bf16 halves
  SBUF/DRAM pressure at no gather cost.
* Edge pass per layer: dst windows of 512 nodes (one PSUM bank each,
  one accumulation group per bank -- a start=True wipes its whole bank,
  so groups must never interleave within a bank). Edges grouped by
  (quad, chunk) with CH=4 table chunks + 1 local self chunk; a single
  fused ACT (bias + relu) flushes PSUM -> hT bf16.
* Per 128-edge tile, the norm-scaled one-hot B[e, d] (bf16 [128, 512])
  is built EITHER on DVE (one 2-op tensor_scalar: is_equal vs f32 iota,
  then mult by norm) OR on the otherwise-idle Scalar engine (two ACTs:
  a = Abs(iota - d); B = Relu(norm - norm*a)); tiles are split between
  the engines to balance load.  One PE matmul (lhsT=gathered M bf16,
  rhs=B) accumulates [H, 512] into the quad PSUM.
* Gather calls are per (quad, chunk) for layers 2/3 and per-quad chops
  for layer 1, chained in issue order on SWDGE queues i%4 (queue index
  must match the tile framework's DMASW lane round-robin).  The whole
  wrapped index array is preloaded per layer.
* Pooling: transpose h3 windows to node-major bf16; indicator matmul
  against batchrel one-hot (bf16) accumulates pooledT [H, 256] in PSUM;
  transpose back to f32 rows, scatter by graph id (indirect DMA, queue
  parity aligned with dummy gathers); AllReduce f32; multiply by 1/cnt.

All structure (tile counts, call sizes) is maxed across cores so the
single SPMD program fits every core; pad slots have norm=0 (B column is
zero) and index 0 (valid row).
"""

import math
from contextlib import ExitStack
from dataclasses import dataclass, field

import numpy as np

import concourse.mybir as mybir
import concourse.tile as tile
from concourse import bacc, bass
from concourse.bass import AP, IndirectOffsetOnAxis, ds
from concourse.masks import make_identity

F32 = mybir.dt.float32
BF16 = mybir.dt.bfloat16
FP16 = mybir.dt.float16
I16 = mybir.dt.int16
I32 = mybir.dt.int32
AF = mybir.ActivationFunctionType
OP = mybir.AluOpType

P = 128  # partitions / hidden size / vocab

DEBUG_STAGE = 0  # 0=off; 1..3 = dump hT after that layer


@dataclass
class Cfg:
    N: int = 100000
    E: int = 1600000
    H: int = 128
    V: int = 128
    L: int = 3
    G: int = 1024
    C: int = 8          # cores
    CH: int = 4         # gather-table chunks (int16 index limit)
    TPC: int = 20       # max tiles per layer-1 dma_gather call
    NQ: int = 4         # SWDGE queues
    SCALAR_FRAC_NUM: int = 3   # of every DEN tiles, NUM go to Scalar
    SCALAR_FRAC_DEN: int = 7

    @property
    def NC(self):
        assert self.N % self.C == 0
        return self.N // self.C

    @property
    def CHN(self):
        assert self.N % self.CH == 0
        return self.N // self.CH

    @property
    def W(self):  # dst windows (128-wide) per core
        return math.ceil(self.NC / P)

    @property
    def Q(self):  # 512-wide window quads per core
        return math.ceil(self.W / 4)

    @property
    def NCP(self):
        return self.W * P

    @property
    def GSPAN(self):
        return 256


@dataclass
class Structure:
    t1_q: list = field(default_factory=list)        # [Q] tiles per quad, l1
    t23_qk: list = field(default_factory=list)      # [Q][CH+1]
    calls1: list = field(default_factory=list)      # [(q, t0, nt)]
    calls23: list = field(default_factory=list)     # [(q, k, t0, nt)]

    @property
    def T1(self):
        return sum(self.t1_q)

    @property
    def T23(self):
        return sum(sum(r) for r in self.t23_qk)


def preprocess(x, edge_index, batch, emb_table, Ws, bs, cfg: Cfg):
    """Host-side (index-only) preprocessing."""
    N, E, C, CH = cfg.N, cfg.E, cfg.C, cfg.CH
    NC, CHN, W, Q = cfg.NC, cfg.CHN, cfg.W, cfg.Q
    CHX = CH + 1

    x = np.asarray(x).astype(np.int64)
    edge_index = np.asarray(edge_index).astype(np.int64)
    batch = np.asarray(batch).astype(np.int64)

    loop = np.arange(N, dtype=np.int64)
    src = np.concatenate([edge_index[0], loop])
    dst = np.concatenate([edge_index[1], loop])
    deg = np.bincount(dst, minlength=N).astype(np.float32)
    dinv = 1.0 / np.sqrt(deg)  # deg >= 1 thanks to self loops
    norm = (dinv[src] * dinv[dst]).astype(np.float32)
    xsrc = x[src]
    dinv2 = (dinv * dinv).astype(np.float32)

    owner = dst // NC

    per_core = []
    for c in range(C):
        m = owner == c
        d_c = dst[m] - c * NC
        o1 = np.argsort(d_c, kind="stable")
        m23 = owner[:E] == c
        s23 = src[:E][m23]
        d23 = dst[:E][m23] - c * NC
        n23 = norm[:E][m23]
        ck23 = s23 // CHN
        vloc = np.arange(NC, dtype=np.int64)
        s23 = np.concatenate([s23, vloc])
        d23 = np.concatenate([d23, vloc])
        n23 = np.concatenate([n23, dinv2[c * NC + vloc]])
        ck23 = np.concatenate([ck23, np.full(NC, CH, np.int64)])
        q23 = d23 // 512
        o23 = np.lexsort((d23, ck23, q23))
        srel23 = np.where(ck23 == CH, s23, s23 - ck23 * CHN)
        per_core.append(dict(
            d=d_c, n=norm[m], xs=xsrc[m], o1=o1,
            s23=srel23, d23=d23, n23=n23, ck23=ck23, o23=o23))

    # ---- uniform tile counts (maxed across cores) ----
    t1_q = np.zeros(Q, dtype=np.int64)
    t23_qk = np.zeros((Q, CHX), dtype=np.int64)
    for c in range(C):
        pc = per_core[c]
        q1 = pc["d"][pc["o1"]] // 512
        cnt1 = np.bincount(q1, minlength=Q)
        t1_q = np.maximum(t1_q, -(-cnt1 // P))
        dk = pc["d23"][pc["o23"]]
        kk = pc["ck23"][pc["o23"]]
        gid = (dk // 512) * CHX + kk
        cntk = np.bincount(gid, minlength=Q * CHX).reshape(Q, CHX)
        t23_qk = np.maximum(t23_qk, -(-cntk // P))
    assert (t1_q >= 1).all()
    assert (t23_qk >= 1).all()

    st = Structure(t1_q=[int(v) for v in t1_q],
                   t23_qk=[list(map(int, r)) for r in t23_qk])

    # ---- call lists ----
    calls1 = []
    toff = 0
    for q in range(Q):
        tq = int(t1_q[q])
        t = 0
        while t < tq:
            nt = min(cfg.TPC, tq - t)
            calls1.append((q, toff + t, nt))
            t += nt
        toff += tq
    st.calls1 = calls1
    calls23 = []
    toff = 0
    for q in range(Q):
        for k in range(CHX):
            tqk = int(t23_qk[q][k])
            calls23.append((q, k, toff, tqk))
            toff += tqk
    st.calls23 = calls23

    # ---- build padded per-core streams ----
    def build_stream(d_sorted, n_sorted, idx_sorted, group_of_edge,
                     counts_T, n_groups):
        """meta [P,T,2] f32 = (dstrel512, norm); metas [P,T,3] f32 =
        (-dstrel512, -norm, norm); idxs [T*128] i16."""
        Ttot = int(sum(counts_T))
        meta = np.zeros((P, Ttot, 2), dtype=np.float32)
        metas = np.zeros((P, Ttot, 3), dtype=np.float32)
        idxs = np.zeros(Ttot * P, dtype=np.int16)
        cnt = np.bincount(group_of_edge, minlength=n_groups)
        starts = np.concatenate([[0], np.cumsum(cnt)[:-1]])
        t0 = 0
        for g in range(n_groups):
            cg, sg, Tg = int(cnt[g]), int(starts[g]), int(counts_T[g])
            assert cg <= Tg * P, (g, cg, Tg)
            sl = slice(sg, sg + cg)
            ii = np.arange(cg)
            tt = t0 + ii // P
            pp = ii % P
            drel = (d_sorted[sl] % 512).astype(np.float32)
            meta[pp, tt, 0] = drel
            meta[pp, tt, 1] = n_sorted[sl]
            metas[pp, tt, 0] = -drel
            metas[pp, tt, 1] = -n_sorted[sl]
            metas[pp, tt, 2] = n_sorted[sl]
            idxs[t0 * P + ii] = idx_sorted[sl].astype(np.int16)
            t0 += Tg
        return meta, metas, idxs

    def wrap_idxs(idx_stream, calls, nidx_cols):
        out = np.zeros((P, nidx_cols), dtype=np.int16)
        col = 0
        for (t0, nt) in calls:
            arr = idx_stream[t0 * P:(t0 + nt) * P]
            wr = arr.reshape(-1, 16).T  # [16, nt*8]
            out[:, col:col + nt * 8] = np.tile(wr, (8, 1))
            col += nt * 8
        assert col == nidx_cols
        return out

    T1, T23 = st.T1, st.T23
    counts1 = [int(t) for t in t1_q]
    counts23 = []
    for q in range(Q):
        for k in range(CHX):
            counts23.append(int(t23_qk[q][k]))

    in_maps = []
    for c in range(C):
        pc = per_core[c]
        o1 = pc["o1"]
        d1, n1, xs1 = pc["d"][o1], pc["n"][o1], pc["xs"][o1]
        g1 = d1 // 512
        meta1, metas1, idx1 = build_stream(d1, n1, xs1, g1, counts1, Q)

        o23 = pc["o23"]
        dk, nk, srel, kk = (pc["d23"][o23], pc["n23"][o23],
                            pc["s23"][o23], pc["ck23"][o23])
        g23 = (dk // 512) * CHX + kk
        assert (np.diff(g23) >= 0).all()
        meta23, metas23, idx23 = build_stream(dk, nk, srel, g23, counts23,
                                              Q * CHX)

        gidx1 = wrap_idxs(idx1, [(t0, nt) for (_q, t0, nt) in st.calls1],
                          T1 * 8)
        gidx23 = wrap_idxs(idx23, [(t0, nt) for (_q, _k, t0, nt)
                                   in st.calls23], T23 * 8)

        # pooling metadata
        nodes = np.arange(cfg.NCP) + c * NC
        valid = nodes < (c + 1) * NC
        bvals = np.where(valid, batch[np.minimum(nodes, N - 1)], -1)
        gmin = int(batch[c * NC])
        gmax = int(batch[min((c + 1) * NC, N) - 1])
        assert gmax - gmin < cfg.GSPAN, (c, gmin, gmax)
        brel = np.where(valid, bvals - gmin, -1).astype(np.float32)
        pool_meta = brel.reshape(cfg.W, P).T.copy()  # [128, W]
        gid_rows = gmin + np.arange(cfg.GSPAN)
        gid_rows = np.where(gid_rows < cfg.G, gid_rows,
                            cfg.G + np.arange(cfg.GSPAN) % 256).astype(np.int32)
        gid_cols = gid_rows.reshape(2, P).T.copy()  # [128, 2]

        cnts = np.bincount(batch, minlength=cfg.G).astype(np.float32)
        recip = 1.0 / np.maximum(cnts, 1.0)
        recip_pm = recip.reshape(cfg.G // P, P).T.copy()

        in_maps.append({
            "meta1": meta1, "metas1": metas1, "gidx1": gidx1,
            "meta23": meta23, "metas23": metas23, "gidx23": gidx23,
            "pool_meta": pool_meta, "gid_cols": gid_cols,
            "recip_pm": recip_pm,
            "emb": np.asarray(emb_table, dtype=np.float32),
            "Ws": np.asarray(Ws, dtype=np.float32),
            "bs": np.asarray(bs, dtype=np.float32),
        })

    return st, in_maps


# --------------------------------------------------------------------------
# device program
# --------------------------------------------------------------------------

def build_nc(cfg: Cfg, st: Structure):
    N, H, C, CH, W, Q = cfg.N, cfg.H, cfg.C, cfg.CH, cfg.W, cfg.Q
    NC, CHN, NCP = cfg.NC, cfg.CHN, cfg.NCP
    CHX = CH + 1
    T1, T23 = st.T1, st.T23
    GS = cfg.GSPAN
    GW = cfg.G // P
    NQ = cfg.NQ

    nc = bacc.Bacc(None, num_devices=C, num_swdge_queues=NQ)
    cores = list(range(C))

    # ---- external I/O ----
    meta1 = nc.declare_dram_parameter("meta1", [P, T1, 2], F32, isOutput=False)
    metas1 = nc.declare_dram_parameter("metas1", [P, T1, 3], F32, isOutput=False)
    gidx1 = nc.declare_dram_parameter("gidx1", [P, T1 * 8], I16, isOutput=False)
    meta23 = nc.declare_dram_parameter("meta23", [P, T23, 2], F32, isOutput=False)
    metas23 = nc.declare_dram_parameter("metas23", [P, T23, 3], F32, isOutput=False)
    gidx23 = nc.declare_dram_parameter("gidx23", [P, T23 * 8], I16, isOutput=False)
    pool_meta = nc.declare_dram_parameter("pool_meta", [P, W], F32, isOutput=False)
    gid_cols = nc.declare_dram_parameter("gid_cols", [P, 2], I32, isOutput=False)
    recip_pm = nc.declare_dram_parameter("recip_pm", [P, GW], F32, isOutput=False)
    emb_d = nc.declare_dram_parameter("emb", [P, H], F32, isOutput=False)
    Ws_d = nc.declare_dram_parameter("Ws", [cfg.L, H, H], F32, isOutput=False)
    bs_d = nc.declare_dram_parameter("bs", [cfg.L, H], F32, isOutput=False)
    out_d = nc.declare_dram_parameter("out", [cfg.G, H], F32, isOutput=True)

    # ---- internal DRAM ----
    t1_dram = nc.dram_tensor("t1_tab", [cfg.V, H], BF16)
    hw_shard = nc.dram_tensor("hw_shard", [NC, H], BF16)
    hw_full = nc.dram_tensor("hw_full", [N, H], BF16, addr_space="Shared")
    pooled_nm = nc.dram_tensor("pooled_nm", [cfg.G + GS, H], F32)
    pooled_sum = nc.dram_tensor("pooled_sum", [cfg.G + GS, H], F32,
                                addr_space="Shared")

    from concourse.tile import add_dep_helper
    pd = {"i": 0, "last": None}

    def chain_pool_dma(inst):
        if pd["last"] is not None:
            add_dep_helper(inst.ins, pd["last"].ins, sync=False,
                           reason="pool-dma queue/lane parity order")
        pd["last"] = inst
        pd["i"] += 1

    with tile.TileContext(nc) as tc, ExitStack() as ctx:
        const = ctx.enter_context(tc.tile_pool(name="const", bufs=1))
        hpool = ctx.enter_context(tc.tile_pool(name="hbuf", bufs=1))

        ident = const.tile([P, P], F32)
        make_identity(nc, ident[:])
        ident_bf = const.tile([P, P], BF16)
        make_identity(nc, ident_bf[:])
        iota_i = const.tile([P, 512], I32)
        nc.gpsimd.iota(iota_i[:], pattern=[[1, 512]], base=0,
                       channel_multiplier=0)
        iota_bf = const.tile([P, GS], BF16)
        nc.vector.tensor_copy(out=iota_bf[:], in_=iota_i[:, :GS])
        iota_f512 = const.tile([P, 512], F32)
        nc.vector.tensor_copy(out=iota_f512[:], in_=iota_i[:])
        iota_v512 = const.tile([P, 512], FP16)
        nc.vector.tensor_copy(out=iota_v512[:], in_=iota_i[:])
        iota_s512 = const.tile([P, 512], FP16)
        nc.vector.tensor_copy(out=iota_s512[:], in_=iota_i[:])
        iota_g512 = const.tile([P, 512], FP16)
        nc.vector.tensor_copy(out=iota_g512[:], in_=iota_i[:])

        w_sb = const.tile([P, H], F32, tag="w_sb")
        w_bf = const.tile([P, H], BF16, tag="w_bf")
        emb_sb = const.tile([P, H], F32, tag="w_sb2")
        b_cols = const.tile([P, cfg.L], F32)
        for l in range(cfg.L):
            nc.sync.dma_start(out=b_cols[:, l:l + 1], in_=bs_d[l, :, None])

        hT_a = hpool.tile([P, NCP], BF16)
        hT_b = hpool.tile([P, NCP], BF16)

        # ---------------- T1 = emb @ W1 (bf16 table) ----------------
        with tc.tile_pool(name="pro", bufs=2) as pro, \
             tc.tile_pool(name="pro_ps", bufs=2, space="PSUM") as pro_ps:
            nc.sync.dma_start(out=emb_sb[:], in_=emb_d[:, :])
            nc.sync.dma_start(out=w_sb[:], in_=Ws_d[0])
            embT_ps = pro_ps.tile([P, P], F32)
            nc.tensor.transpose(out=embT_ps[:], in_=emb_sb[:], identity=ident[:])
            embT = pro.tile([P, P], F32)
            nc.vector.tensor_copy(out=embT[:], in_=embT_ps[:])
            t1t_ps = pro_ps.tile([P, P], F32)
            nc.tensor.matmul(out=t1t_ps[:], lhsT=w_sb[:], rhs=embT[:],
                             start=True, stop=True)
            t1t = pro.tile([P, P], F32)
            nc.vector.tensor_copy(out=t1t[:], in_=t1t_ps[:])
            t1nm_ps = pro_ps.tile([P, P], F32)
            nc.tensor.transpose(out=t1nm_ps[:], in_=t1t[:], identity=ident[:])
            t1nm = pro.tile([P, P], BF16)
            nc.vector.tensor_copy(out=t1nm[:], in_=t1nm_ps[:])
            nc.sync.dma_start(out=t1_dram[:, :], in_=t1nm[:])

        # ---------------- edge pass ----------------
        tilectr = [0]

        def edge_pass(layer, h_out):
            l1 = layer == 0
            meta_d = meta1 if l1 else meta23
            metas_d = metas1 if l1 else metas23
            gidx_d = gidx1 if l1 else gidx23
            calls = ([(q, None, t0, nt) for (q, t0, nt) in st.calls1] if l1
                     else st.calls23)
            Ttot = T1 if l1 else T23
            MAXT = max(c[3] for c in calls)

            with tc.tile_pool(name=f"ix{layer}", bufs=1) as ixp, \
                 tc.tile_pool(name=f"ep{layer}", bufs=8) as ep, \
                 tc.tile_pool(name=f"gb{layer}", bufs=14) as gb, \
                 tc.tile_pool(name=f"bq{layer}", bufs=6) as bq, \
                 tc.tile_pool(name=f"eps{layer}", bufs=2, space="PSUM") as eps:

                # preload the whole wrapped index array
                idxall = ixp.tile([P, Ttot * 8], I16, tag="idxall")
                IXC = 8192
                for s0 in range(0, Ttot * 8, IXC):
                    nn = min(IXC, Ttot * 8 - s0)
                    nc.sync.dma_start(out=idxall[:, s0:s0 + nn],
                                      in_=gidx_d[:, s0:s0 + nn])

                gbuf, mbuf, msbuf = {}, {}, {}
                idxcol = 0
                for ci, (q, k, t0, nt) in enumerate(calls):
                    mb = ep.tile([P, MAXT, 2], F32, tag="meta")
                    nc.sync.dma_start(out=mb[:, :nt, :],
                                      in_=meta_d[:, t0:t0 + nt, :])
                    mbs = ep.tile([P, MAXT, 3], F32, tag="metas")
                    nc.sync.dma_start(out=mbs[:, :nt, :],
                                      in_=metas_d[:, t0:t0 + nt, :])
                    g = gb.tile([P, MAXT, H], BF16, tag="gath")
                    if l1:
                        src_ap = t1_dram[:, :]
                    elif k == CH:
                        src_ap = hw_shard[:, :]
                    else:
                        src_ap = hw_full[k * CHN:(k + 1) * CHN, :]
                    # split into NQ sub-calls on all queues; they share one
                    # buffer (disjoint slices) so their waits are identical
                    # and they dispatch together -> the SWDGE ucode batches
                    # them across queue engines (~4x descriptor rate)
                    nsub = min(NQ, nt)
                    per = -(-nt // nsub)
                    s0 = 0
                    while s0 < nt:
                        sn = min(per, nt - s0)
                        gi = nc.gpsimd.dma_gather(
                            out_ap=g[:, s0:s0 + sn, :], in_ap=src_ap,
                            idxs_ap=idxall[:, idxcol + s0 * 8:
                                           idxcol + (s0 + sn) * 8],
                            num_idxs=sn * P, num_idxs_reg=sn * P,
                            elem_size=H, single_packet=False,
                            queue_num=pd["i"] % NQ)
                        chain_pool_dma(gi)
                        s0 += sn
                    gbuf[ci] = g
                    mbuf[ci] = mb
                    msbuf[ci] = mbs
                    idxcol += nt * 8

                def emit_tile(ci, slot, qpsum, first, last):
                    g = gbuf[ci]
                    Bfull = bq.tile([P, 1024], BF16, tag="B")
                    B = Bfull[:, :512]
                    tc_ = tilectr[0]
                    tilectr[0] += 1
                    r9 = tc_ % 9
                    if r9 == 8:
                        mb = mbuf[ci]
                        nc.gpsimd.tensor_scalar(
                            out=B[:], in0=iota_g512[:],
                            scalar1=mb[:, slot, 0:1],
                            scalar2=mb[:, slot, 1:2],
                            op0=OP.is_equal, op1=OP.mult)
                    elif r9 >= 5:
                        mbs = msbuf[ci]
                        afull = bq.tile([P, 1024], BF16, tag="A")
                        a = afull[:, :512]
                        nc.scalar.activation(
                            out=a[:], in_=iota_s512[:], func=AF.Abs,
                            bias=mbs[:, slot, 0:1], scale=1.0)
                        nc.scalar.activation(
                            out=B[:], in_=a[:], func=AF.Relu,
                            bias=mbs[:, slot, 2:3],
                            scale=mbs[:, slot, 1:2])
                    else:
                        mb = mbuf[ci]
                        nc.vector.tensor_scalar(
                            out=B[:], in0=iota_v512[:],
                            scalar1=mb[:, slot, 0:1],
                            scalar2=mb[:, slot, 1:2],
                            op0=OP.is_equal, op1=OP.mult)
                    nc.tensor.matmul(
                        out=qpsum[:], lhsT=g[:, slot, :], rhs=B[:],
                        start=first, stop=last)

                ci_of = {}
                for ci, cl in enumerate(calls):
                    ci_of.setdefault((cl[0], cl[1]), []).append(ci)

                func = AF.Relu if layer < cfg.L - 1 else AF.Identity
                for q in range(Q):
                    nw = min(4, W - q * 4)
                    qpsum = eps.tile([P, 512], F32, tag="qp")
                    if l1:
                        tq = st.t1_q[q]
                        cis = ci_of[(q, None)]
                        sizes = [calls[ci][3] for ci in cis]
                        for i in range(tq):
                            rem = i
                            for ci, sz in zip(cis, sizes):
                                if rem < sz:
                                    break
                                rem -= sz
                            emit_tile(ci, rem, qpsum, i == 0, i == tq - 1)
                    else:
                        tq = sum(st.t23_qk[q])
                        done = 0
                        for k in range(CHX):
                            (ci,) = ci_of[(q, k)]
                            for i in range(st.t23_qk[q][k]):
                                emit_tile(ci, i, qpsum,
                                          done == 0, done == tq - 1)
                                done += 1
                    ncol = nw * P
                    nc.scalar.activation(
                        out=h_out[:, q * 512:q * 512 + ncol],
                        in_=qpsum[:, :ncol], func=func,
                        bias=b_cols[:, layer:layer + 1], scale=1.0)

        # ---------------- hw phase ----------------
        def hw_phase(layer, h_in):
            with tc.tile_pool(name=f"hw{layer}", bufs=3) as hp, \
                 tc.tile_pool(name=f"hwps{layer}", bufs=2, space="PSUM") as hps, \
                 tc.tile_pool(name=f"hwps2{layer}", bufs=2, space="PSUM") as hps2:
                nc.sync.dma_start(out=w_sb[:], in_=Ws_d[layer])
                nc.vector.tensor_copy(out=w_bf[:], in_=w_sb[:])
                CWW = 512
                for j0 in range(0, NC, CWW):
                    nj = min(CWW, NC - j0)
                    ps = hps.tile([P, CWW], F32, tag="mm")
                    nc.tensor.matmul(out=ps[:, :nj], lhsT=w_bf[:],
                                     rhs=h_in[:, j0:j0 + nj],
                                     start=True, stop=True)
                    hw_s = hp.tile([P, CWW], F32, tag="hw_s")
                    nc.scalar.activation(out=hw_s[:, :nj], in_=ps[:, :nj],
                                         func=AF.Copy)
                    for q0 in range(0, nj, P):
                        nq = min(P, nj - q0)
                        pt = hps2.tile([P, P], F32, tag="tr")
                        nc.tensor.transpose(out=pt[:nq, :],
                                            in_=hw_s[:, q0:q0 + nq],
                                            identity=ident[:])
                        stg = hp.tile([P, P], BF16, tag="stg")
                        nc.scalar.activation(out=stg[:nq, :], in_=pt[:nq, :],
                                             func=AF.Copy)
                        nc.sync.dma_start(
                            out=hw_shard[j0 + q0:j0 + q0 + nq, :],
                            in_=stg[:nq, :])
            nc.gpsimd.collective_compute(
                "AllGather", OP.bypass, replica_groups=[cores],
                ins=[hw_shard[:, :]], outs=[hw_full[:, :]])

        def dump_h(src_tile):
            dbg_d = nc.declare_dram_parameter("dbg", [P, NCP], F32,
                                              isOutput=True)
            with tc.tile_pool(name="dbg", bufs=2) as dbp:
                CWD = 512
                for s0 in range(0, NCP, CWD):
                    nn = min(CWD, NCP - s0)
                    dt_ = dbp.tile([P, CWD], F32, tag="d")
                    nc.vector.tensor_copy(out=dt_[:, :nn],
                                          in_=src_tile[:, s0:s0 + nn])
                    nc.sync.dma_start(out=dbg_d[:, s0:s0 + nn],
                                      in_=dt_[:, :nn])

        with nc.named_scope("layer1"):
            edge_pass(0, hT_a)
        if DEBUG_STAGE == 1:
            dump_h(hT_a)
        with nc.named_scope("hw2"):
            hw_phase(1, hT_a)
        with nc.named_scope("layer2"):
            edge_pass(1, hT_b)
        if DEBUG_STAGE == 2:
            dump_h(hT_b)
        with nc.named_scope("hw3"):
            hw_phase(2, hT_b)
        with nc.named_scope("layer3"):
            edge_pass(2, hT_a)
        if DEBUG_STAGE == 3:
            dump_h(hT_a)

        # ---------------- pooling ----------------
        with nc.named_scope("pool"), \
             tc.tile_pool(name="po", bufs=3) as po, \
             tc.tile_pool(name="po_ps", bufs=2, space="PSUM") as po_ps, \
             tc.tile_pool(name="po_acc", bufs=1, space="PSUM") as po_acc:
            pm = po.tile([P, W], F32, tag="pm")
            nc.sync.dma_start(out=pm[:], in_=pool_meta[:, :])
            gcols = po.tile([P, 2], I32, tag="gcols")
            nc.sync.dma_start(out=gcols[:], in_=gid_cols[:, :])
            recip_sb = po.tile([P, GW], F32, tag="recip")
            nc.sync.dma_start(out=recip_sb[:], in_=recip_pm[:, :])

            acc = po_acc.tile([P, GS], F32)
            for t in range(W):
                pt = po_ps.tile([P, P], BF16, tag="ptr")
                nc.tensor.transpose(out=pt[:], in_=hT_a[:, t * P:(t + 1) * P],
                                    identity=ident_bf[:])
                h3nm = po.tile([P, P], BF16, tag="h3nm")
                nc.scalar.activation(out=h3nm[:], in_=pt[:], func=AF.Copy)
                Bp = po.tile([P, GS], BF16, tag="Bp")
                nc.vector.tensor_scalar(
                    out=Bp[:], in0=iota_bf[:],
                    scalar1=pm[:, t:t + 1], scalar2=None,
                    op0=OP.is_equal)
                nc.tensor.matmul(out=acc[:], lhsT=h3nm[:], rhs=Bp[:],
                                 start=(t == 0), stop=(t == W - 1))

            def dummy_gather():
                dz = po.tile([P, 1, P], BF16, tag="dz")
                zi = po.tile([P, 8], I16, tag="zi")
                nc.vector.memset(zi[:], 0)
                gi = nc.gpsimd.dma_gather(
                    out_ap=dz[:], in_ap=t1_dram[:, :], idxs_ap=zi[:],
                    num_idxs=P, num_idxs_reg=P, elem_size=H,
                    single_packet=False, queue_num=pd["i"] % NQ)
                chain_pool_dma(gi)

            zt = po.tile([P, P], F32, tag="zt")
            nc.vector.memset(zt[:], 0.0)
            for r0 in range(0, cfg.G + GS, P):
                nc.sync.dma_start(out=pooled_nm[r0:r0 + P, :], in_=zt[:])

            acc_sb = po.tile([P, GS], F32, tag="acc_sb")
            nc.scalar.activation(out=acc_sb[:], in_=acc[:], func=AF.Copy)
            for half in range(2):
                pt = po_ps.tile([P, P], F32, tag="ptr2")
                nc.tensor.transpose(out=pt[:],
                                    in_=acc_sb[:, half * P:(half + 1) * P],
                                    identity=ident[:])
                rows = po.tile([P, P], F32, tag="rows")
                nc.scalar.activation(out=rows[:], in_=pt[:], func=AF.Copy)
                while pd["i"] % NQ != 0:
                    dummy_gather()  # scatters run on queue 0: align lane
                si = nc.gpsimd.indirect_dma_start(
                    out=pooled_nm[:, :],
                    out_offset=IndirectOffsetOnAxis(
                        ap=gcols[:, half:half + 1], axis=0),
                    in_=rows[:], in_offset=None)
                chain_pool_dma(si)

            nc.gpsimd.collective_compute(
                "AllReduce", OP.add, replica_groups=[cores],
                ins=[pooled_nm[:, :]], outs=[pooled_sum[:, :]])

            for gw in range(GW):
                ot = po.tile([P, H], F32, tag="ot")
                nc.sync.dma_start(out=ot[:],
                                  in_=pooled_sum[gw * P:(gw + 1) * P, :])
                os = po.tile([P, H], F32, tag="os")
                nc.vector.tensor_scalar(
                    out=os[:], in0=ot[:], scalar1=recip_sb[:, gw:gw + 1],
                    scalar2=None, op0=OP.mult)
                nc.sync.dma_start(out=out_d[gw * P:(gw + 1) * P, :],
                                  in_=os[:])

    return nc


# --------------------------------------------------------------------------
# entry point: full inputs -> full output
# --------------------------------------------------------------------------

_CACHE = {}


def _get_compiled(cfg, st_key, st):
    if st_key not in _CACHE:
        nc = build_nc(cfg, st)
        nc.finalize()
        _CACHE[st_key] = nc
    return _CACHE[st_key]


def kernel(x, edge_index, batch, emb_table, Ws, bs):
    cfg = Cfg()  # full problem size, hardcoded
    st, in_maps = preprocess(x, edge_index, batch, emb_table, Ws, bs, cfg)
    st_key = (tuple(st.t1_q), tuple(tuple(r) for r in st.t23_qk))
    nc = _get_compiled(cfg, st_key, st)

    from concourse.bass_utils import run_bass_kernel_spmd

    res = run_bass_kernel_spmd(nc, in_maps, list(range(cfg.C)))
    return np.ascontiguousarray(res.results[0]["out"])


# revision 16
# speedup vs baseline: 2.0962x; 2.0962x over previous
"""GCN embedder kernel for TRN2, 8-core SPMD (v5: bf16, 4 SWDGE queues,
512-wide quad one-hots, DVE+Scalar split one-hot builds).

Design
------
* Nodes sharded contiguously across C=8 cores (NC nodes each). Edges
  (incl. self-loops) are owned by the dst core.
* Node features h are kept feature-major in SBUF as bf16: hT [H=128, NCP].
* Gather tables (T1 = emb@W1 for layer 1, hw_full = h@W_l for layers 2/3)
  are bf16 in DRAM; dma_gather cost is purely per-index, so bf16 halves
  SBUF/DRAM pressure at no gather cost.
* Edge pass per layer: dst windows of 512 nodes (one PSUM bank each,
  one accumulation group per bank -- a start=True wipes its whole bank,
  so groups must never interleave within a bank). Edges grouped by
  (quad, chunk) with CH=4 table chunks + 1 local self chunk; a single
  fused ACT (bias + relu) flushes PSUM -> hT bf16.
* Per 128-edge tile, the norm-scaled one-hot B[e, d] (bf16 [128, 512])
  is built EITHER on DVE (one 2-op tensor_scalar: is_equal vs f32 iota,
  then mult by norm) OR on the otherwise-idle Scalar engine (two ACTs:
  a = Abs(iota - d); B = Relu(norm - norm*a)); tiles are split between
  the engines to balance load.  One PE matmul (lhsT=gathered M bf16,
  rhs=B) accumulates [H, 512] into the quad PSUM.
* Gather calls are per (quad, chunk) for layers 2/3 and per-quad chops
  for layer 1, chained in issue order on SWDGE queues i%4 (queue index
  must match the tile framework's DMASW lane round-robin).  The whole
  wrapped index array is preloaded per layer.
* Pooling: transpose h3 windows to node-major bf16; indicator matmul
  against batchrel one-hot (bf16) accumulates pooledT [H, 256] in PSUM;
  transpose back to f32 rows, scatter by graph id (indirect DMA, queue
  parity aligned with dummy gathers); AllReduce f32; multiply by 1/cnt.

All structure (tile counts, call sizes) is maxed across cores so the
single SPMD program fits every core; pad slots have norm=0 (B column is
zero) and index 0 (valid row).
"""

import math
from contextlib import ExitStack
from dataclasses import dataclass, field

import numpy as np

import concourse.mybir as mybir
import concourse.tile as tile
from concourse import bacc, bass
from concourse.bass import AP, IndirectOffsetOnAxis, ds
from concourse.masks import make_identity

F32 = mybir.dt.float32
BF16 = mybir.dt.bfloat16
FP16 = mybir.dt.float16
I16 = mybir.dt.int16
I32 = mybir.dt.int32
AF = mybir.ActivationFunctionType
OP = mybir.AluOpType

P = 128  # partitions / hidden size / vocab

DEBUG_STAGE = 0  # 0=off; 1..3 = dump hT after that layer


@dataclass
class Cfg:
    N: int = 100000
    E: int = 1600000
    H: int = 128
    V: int = 128
    L: int = 3
    G: int = 1024
    C: int = 8          # cores
    CH: int = 4         # gather-table chunks (int16 index limit)
    TPC: int = 20       # max tiles per layer-1 dma_gather call
    NQ: int = 4         # SWDGE queues
    SCALAR_FRAC_NUM: int = 3   # of every DEN tiles, NUM go to Scalar
    SCALAR_FRAC_DEN: int = 7

    @property
    def NC(self):
        assert self.N % self.C == 0
        return self.N // self.C

    @property
    def CHN(self):
        assert self.N % self.CH == 0
        return self.N // self.CH

    @property
    def W(self):  # dst windows (128-wide) per core
        return math.ceil(self.NC / P)

    @property
    def Q(self):  # 512-wide window quads per core
        return math.ceil(self.W / 4)

    @property
    def NCP(self):
        return self.W * P

    @property
    def GSPAN(self):
        return 256


@dataclass
class Structure:
    t1_q: list = field(default_factory=list)        # [Q] tiles per quad, l1
    t23_qk: list = field(default_factory=list)      # [Q][CH+1]
    calls1: list = field(default_factory=list)      # [(q, t0, nt)]
    calls23: list = field(default_factory=list)     # [(q, k, t0, nt)]

    @property
    def T1(self):
        return sum(self.t1_q)

    @property
    def T23(self):
        return sum(sum(r) for r in self.t23_qk)


def preprocess(x, edge_index, batch, emb_table, Ws, bs, cfg: Cfg):
    """Host-side (index-only) preprocessing."""
    N, E, C, CH = cfg.N, cfg.E, cfg.C, cfg.CH
    NC, CHN, W, Q = cfg.NC, cfg.CHN, cfg.W, cfg.Q
    CHX = CH + 1

    x = np.asarray(x).astype(np.int64)
    edge_index = np.asarray(edge_index).astype(np.int64)
    batch = np.asarray(batch).astype(np.int64)

    loop = np.arange(N, dtype=np.int64)
    src = np.concatenate([edge_index[0], loop])
    dst = np.concatenate([edge_index[1], loop])
    deg = np.bincount(dst, minlength=N).astype(np.float32)
    dinv = 1.0 / np.sqrt(deg)  # deg >= 1 thanks to self loops
    norm = (dinv[src] * dinv[dst]).astype(np.float32)
    xsrc = x[src]
    dinv2 = (dinv * dinv).astype(np.float32)

    owner = dst // NC

    per_core = []
    for c in range(C):
        m = owner == c
        d_c = dst[m] - c * NC
        o1 = np.argsort(d_c, kind="stable")
        m23 = owner[:E] == c
        s23 = src[:E][m23]
        d23 = dst[:E][m23] - c * NC
        n23 = norm[:E][m23]
        ck23 = s23 // CHN
        vloc = np.arange(NC, dtype=np.int64)
        s23 = np.concatenate([s23, vloc])
        d23 = np.concatenate([d23, vloc])
        n23 = np.concatenate([n23, dinv2[c * NC + vloc]])
        ck23 = np.concatenate([ck23, np.full(NC, CH, np.int64)])
        q23 = d23 // 512
        o23 = np.lexsort((d23, ck23, q23))
        srel23 = np.where(ck23 == CH, s23, s23 - ck23 * CHN)
        per_core.append(dict(
            d=d_c, n=norm[m], xs=xsrc[m], o1=o1,
            s23=srel23, d23=d23, n23=n23, ck23=ck23, o23=o23))

    # ---- uniform tile counts (maxed across cores) ----
    t1_q = np.zeros(Q, dtype=np.int64)
    t23_qk = np.zeros((Q, CHX), dtype=np.int64)
    for c in range(C):
        pc = per_core[c]
        q1 = pc["d"][pc["o1"]] // 512
        cnt1 = np.bincount(q1, minlength=Q)
        t1_q = np.maximum(t1_q, -(-cnt1 // P))
        dk = pc["d23"][pc["o23"]]
        kk = pc["ck23"][pc["o23"]]
        gid = (dk // 512) * CHX + kk
        cntk = np.bincount(gid, minlength=Q * CHX).reshape(Q, CHX)
        t23_qk = np.maximum(t23_qk, -(-cntk // P))
    assert (t1_q >= 1).all()
    assert (t23_qk >= 1).all()

    st = Structure(t1_q=[int(v) for v in t1_q],
                   t23_qk=[list(map(int, r)) for r in t23_qk])

    # ---- call lists ----
    calls1 = []
    toff = 0
    for q in range(Q):
        tq = int(t1_q[q])
        t = 0
        while t < tq:
            nt = min(cfg.TPC, tq - t)
            calls1.append((q, toff + t, nt))
            t += nt
        toff += tq
    st.calls1 = calls1
    calls23 = []
    toff = 0
    for q in range(Q):
        for k in range(CHX):
            tqk = int(t23_qk[q][k])
            calls23.append((q, k, toff, tqk))
            toff += tqk
    st.calls23 = calls23

    # ---- build padded per-core streams ----
    def build_stream(d_sorted, n_sorted, idx_sorted, group_of_edge,
                     counts_T, n_groups):
        """meta [P,T,2] f32 = (dstrel512, norm); metas [P,T,3] f32 =
        (-dstrel512, -norm, norm); idxs [T*128] i16."""
        Ttot = int(sum(counts_T))
        meta = np.zeros((P, Ttot, 2), dtype=np.float32)
        metas = np.zeros((P, Ttot, 3), dtype=np.float32)
        idxs = np.zeros(Ttot * P, dtype=np.int16)
        cnt = np.bincount(group_of_edge, minlength=n_groups)
        starts = np.concatenate([[0], np.cumsum(cnt)[:-1]])
        t0 = 0
        for g in range(n_groups):
            cg, sg, Tg = int(cnt[g]), int(starts[g]), int(counts_T[g])
            assert cg <= Tg * P, (g, cg, Tg)
            sl = slice(sg, sg + cg)
            ii = np.arange(cg)
            tt = t0 + ii // P
            pp = ii % P
            drel = (d_sorted[sl] % 512).astype(np.float32)
            meta[pp, tt, 0] = drel
            meta[pp, tt, 1] = n_sorted[sl]
            metas[pp, tt, 0] = -drel
            metas[pp, tt, 1] = -n_sorted[sl]
            metas[pp, tt, 2] = n_sorted[sl]
            idxs[t0 * P + ii] = idx_sorted[sl].astype(np.int16)
            t0 += Tg
        return meta, metas, idxs

    def wrap_idxs(idx_stream, calls, nidx_cols):
        out = np.zeros((P, nidx_cols), dtype=np.int16)
        col = 0
        for (t0, nt) in calls:
            arr = idx_stream[t0 * P:(t0 + nt) * P]
            wr = arr.reshape(-1, 16).T  # [16, nt*8]
            out[:, col:col + nt * 8] = np.tile(wr, (8, 1))
            col += nt * 8
        assert col == nidx_cols
        return out

    T1, T23 = st.T1, st.T23
    counts1 = [int(t) for t in t1_q]
    counts23 = []
    for q in range(Q):
        for k in range(CHX):
            counts23.append(int(t23_qk[q][k]))

    in_maps = []
    for c in range(C):
        pc = per_core[c]
        o1 = pc["o1"]
        d1, n1, xs1 = pc["d"][o1], pc["n"][o1], pc["xs"][o1]
        g1 = d1 // 512
        meta1, metas1, idx1 = build_stream(d1, n1, xs1, g1, counts1, Q)

        o23 = pc["o23"]
        dk, nk, srel, kk = (pc["d23"][o23], pc["n23"][o23],
                            pc["s23"][o23], pc["ck23"][o23])
        g23 = (dk // 512) * CHX + kk
        assert (np.diff(g23) >= 0).all()
        meta23, metas23, idx23 = build_stream(dk, nk, srel, g23, counts23,
                                              Q * CHX)

        gidx1 = wrap_idxs(idx1, [(t0, nt) for (_q, t0, nt) in st.calls1],
                          T1 * 8)
        gidx23 = wrap_idxs(idx23, [(t0, nt) for (_q, _k, t0, nt)
                                   in st.calls23], T23 * 8)

        # pooling metadata
        nodes = np.arange(cfg.NCP) + c * NC
        valid = nodes < (c + 1) * NC
        bvals = np.where(valid, batch[np.minimum(nodes, N - 1)], -1)
        gmin = int(batch[c * NC])
        gmax = int(batch[min((c + 1) * NC, N) - 1])
        assert gmax - gmin < cfg.GSPAN, (c, gmin, gmax)
        brel = np.where(valid, bvals - gmin, -1).astype(np.float32)
        pool_meta = brel.reshape(cfg.W, P).T.copy()  # [128, W]
        gid_rows = gmin + np.arange(cfg.GSPAN)
        gid_rows = np.where(gid_rows < cfg.G, gid_rows,
                            cfg.G + np.arange(cfg.GSPAN) % 256).astype(np.int32)
        gid_cols = gid_rows.reshape(2, P).T.copy()  # [128, 2]

        cnts = np.bincount(batch, minlength=cfg.G).astype(np.float32)
        recip = 1.0 / np.maximum(cnts, 1.0)
        recip_pm = recip.reshape(cfg.G // P, P).T.copy()

        in_maps.append({
            "meta1": meta1, "metas1": metas1, "gidx1": gidx1,
            "meta23": meta23, "metas23": metas23, "gidx23": gidx23,
            "pool_meta": pool_meta, "gid_cols": gid_cols,
            "recip_pm": recip_pm,
            "emb": np.asarray(emb_table, dtype=np.float32),
            "Ws": np.asarray(Ws, dtype=np.float32),
            "bs": np.asarray(bs, dtype=np.float32),
        })

    return st, in_maps


# --------------------------------------------------------------------------
# device program
# --------------------------------------------------------------------------

def build_nc(cfg: Cfg, st: Structure):
    N, H, C, CH, W, Q = cfg.N, cfg.H, cfg.C, cfg.CH, cfg.W, cfg.Q
    NC, CHN, NCP = cfg.NC, cfg.CHN, cfg.NCP
    CHX = CH + 1
    T1, T23 = st.T1, st.T23
    GS = cfg.GSPAN
    GW = cfg.G // P
    NQ = cfg.NQ

    nc = bacc.Bacc(None, num_devices=C, num_swdge_queues=NQ)
    cores = list(range(C))

    # ---- external I/O ----
    meta1 = nc.declare_dram_parameter("meta1", [P, T1, 2], F32, isOutput=False)
    metas1 = nc.declare_dram_parameter("metas1", [P, T1, 3], F32, isOutput=False)
    gidx1 = nc.declare_dram_parameter("gidx1", [P, T1 * 8], I16, isOutput=False)
    meta23 = nc.declare_dram_parameter("meta23", [P, T23, 2], F32, isOutput=False)
    metas23 = nc.declare_dram_parameter("metas23", [P, T23, 3], F32, isOutput=False)
    gidx23 = nc.declare_dram_parameter("gidx23", [P, T23 * 8], I16, isOutput=False)
    pool_meta = nc.declare_dram_parameter("pool_meta", [P, W], F32, isOutput=False)
    gid_cols = nc.declare_dram_parameter("gid_cols", [P, 2], I32, isOutput=False)
    recip_pm = nc.declare_dram_parameter("recip_pm", [P, GW], F32, isOutput=False)
    emb_d = nc.declare_dram_parameter("emb", [P, H], F32, isOutput=False)
    Ws_d = nc.declare_dram_parameter("Ws", [cfg.L, H, H], F32, isOutput=False)
    bs_d = nc.declare_dram_parameter("bs", [cfg.L, H], F32, isOutput=False)
    out_d = nc.declare_dram_parameter("out", [cfg.G, H], F32, isOutput=True)

    # ---- internal DRAM ----
    t1_dram = nc.dram_tensor("t1_tab", [cfg.V, H], BF16)
    hw_shard = nc.dram_tensor("hw_shard", [NC, H], BF16)
    hw_full = nc.dram_tensor("hw_full", [N, H], BF16, addr_space="Shared")
    pooled_nm = nc.dram_tensor("pooled_nm", [cfg.G + GS, H], F32)
    pooled_sum = nc.dram_tensor("pooled_sum", [cfg.G + GS, H], F32,
                                addr_space="Shared")

    from concourse.tile import add_dep_helper
    pd = {"i": 0, "last": None}

    def chain_pool_dma(inst):
        if pd["last"] is not None:
            add_dep_helper(inst.ins, pd["last"].ins, sync=False,
                           reason="pool-dma queue/lane parity order")
        pd["last"] = inst
        pd["i"] += 1

    with tile.TileContext(nc) as tc, ExitStack() as ctx:
        const = ctx.enter_context(tc.tile_pool(name="const", bufs=1))
        hpool = ctx.enter_context(tc.tile_pool(name="hbuf", bufs=1))

        ident = const.tile([P, P], F32)
        make_identity(nc, ident[:])
        ident_bf = const.tile([P, P], BF16)
        make_identity(nc, ident_bf[:])
        iota_i = const.tile([P, 512], I32)
        nc.gpsimd.iota(iota_i[:], pattern=[[1, 512]], base=0,
                       channel_multiplier=0)
        iota_bf = const.tile([P, GS], BF16)
        nc.vector.tensor_copy(out=iota_bf[:], in_=iota_i[:, :GS])
        iota_f512 = const.tile([P, 512], F32)
        nc.vector.tensor_copy(out=iota_f512[:], in_=iota_i[:])
        iota_v512 = const.tile([P, 512], F32)
        nc.vector.tensor_copy(out=iota_v512[:], in_=iota_i[:])
        iota_s512 = const.tile([P, 512], F32)
        nc.vector.tensor_copy(out=iota_s512[:], in_=iota_i[:])

        w_sb = const.tile([P, H], F32, tag="w_sb")
        w_bf = const.tile([P, H], BF16, tag="w_bf")
        emb_sb = const.tile([P, H], F32, tag="w_sb2")
        b_cols = const.tile([P, cfg.L], F32)
        for l in range(cfg.L):
            nc.sync.dma_start(out=b_cols[:, l:l + 1], in_=bs_d[l, :, None])

        hT_a = hpool.tile([P, NCP], BF16)
        hT_b = hpool.tile([P, NCP], BF16)

        # ---------------- T1 = emb @ W1 (bf16 table) ----------------
        with tc.tile_pool(name="pro", bufs=2) as pro, \
             tc.tile_pool(name="pro_ps", bufs=2, space="PSUM") as pro_ps:
            nc.sync.dma_start(out=emb_sb[:], in_=emb_d[:, :])
            nc.sync.dma_start(out=w_sb[:], in_=Ws_d[0])
            embT_ps = pro_ps.tile([P, P], F32)
            nc.tensor.transpose(out=embT_ps[:], in_=emb_sb[:], identity=ident[:])
            embT = pro.tile([P, P], F32)
            nc.vector.tensor_copy(out=embT[:], in_=embT_ps[:])
            t1t_ps = pro_ps.tile([P, P], F32)
            nc.tensor.matmul(out=t1t_ps[:], lhsT=w_sb[:], rhs=embT[:],
                             start=True, stop=True)
            t1t = pro.tile([P, P], F32)
            nc.vector.tensor_copy(out=t1t[:], in_=t1t_ps[:])
            t1nm_ps = pro_ps.tile([P, P], F32)
            nc.tensor.transpose(out=t1nm_ps[:], in_=t1t[:], identity=ident[:])
            t1nm = pro.tile([P, P], BF16)
            nc.vector.tensor_copy(out=t1nm[:], in_=t1nm_ps[:])
            nc.sync.dma_start(out=t1_dram[:, :], in_=t1nm[:])

        # ---------------- edge pass ----------------
        tilectr = [0]

        def edge_pass(layer, h_out):
            l1 = layer == 0
            meta_d = meta1 if l1 else meta23
            metas_d = metas1 if l1 else metas23
            gidx_d = gidx1 if l1 else gidx23
            calls = ([(q, None, t0, nt) for (q, t0, nt) in st.calls1] if l1
                     else st.calls23)
            Ttot = T1 if l1 else T23
            MAXT = max(c[3] for c in calls)

            with tc.tile_pool(name=f"ix{layer}", bufs=1) as ixp, \
                 tc.tile_pool(name=f"ep{layer}", bufs=8) as ep, \
                 tc.tile_pool(name=f"gb{layer}", bufs=14) as gb, \
                 tc.tile_pool(name=f"bq{layer}", bufs=6) as bq, \
                 tc.tile_pool(name=f"eps{layer}", bufs=2, space="PSUM") as eps:

                # preload the whole wrapped index array
                idxall = ixp.tile([P, Ttot * 8], I16, tag="idxall")
                IXC = 8192
                for s0 in range(0, Ttot * 8, IXC):
                    nn = min(IXC, Ttot * 8 - s0)
                    nc.sync.dma_start(out=idxall[:, s0:s0 + nn],
                                      in_=gidx_d[:, s0:s0 + nn])

                gbuf, mbuf, msbuf = {}, {}, {}
                idxcol = 0
                for ci, (q, k, t0, nt) in enumerate(calls):
                    mb = ep.tile([P, MAXT, 2], F32, tag="meta")
                    nc.sync.dma_start(out=mb[:, :nt, :],
                                      in_=meta_d[:, t0:t0 + nt, :])
                    mbs = ep.tile([P, MAXT, 3], F32, tag="metas")
                    nc.sync.dma_start(out=mbs[:, :nt, :],
                                      in_=metas_d[:, t0:t0 + nt, :])
                    g = gb.tile([P, MAXT, H], BF16, tag="gath")
                    if l1:
                        src_ap = t1_dram[:, :]
                    elif k == CH:
                        src_ap = hw_shard[:, :]
                    else:
                        src_ap = hw_full[k * CHN:(k + 1) * CHN, :]
                    # split into NQ sub-calls on all queues; they share one
                    # buffer (disjoint slices) so their waits are identical
                    # and they dispatch together -> the SWDGE ucode batches
                    # them across queue engines (~4x descriptor rate)
                    nsub = min(NQ, nt)
                    per = -(-nt // nsub)
                    s0 = 0
                    while s0 < nt:
                        sn = min(per, nt - s0)
                        gi = nc.gpsimd.dma_gather(
                            out_ap=g[:, s0:s0 + sn, :], in_ap=src_ap,
                            idxs_ap=idxall[:, idxcol + s0 * 8:
                                           idxcol + (s0 + sn) * 8],
                            num_idxs=sn * P, num_idxs_reg=sn * P,
                            elem_size=H, single_packet=False,
                            queue_num=pd["i"] % NQ)
                        chain_pool_dma(gi)
                        s0 += sn
                    gbuf[ci] = g
                    mbuf[ci] = mb
                    msbuf[ci] = mbs
                    idxcol += nt * 8

                def emit_tile(ci, slot, qpsum, first, last):
                    g = gbuf[ci]
                    Bfull = bq.tile([P, 1024], BF16, tag="B")
                    B = Bfull[:, :512]
                    tc_ = tilectr[0]
                    tilectr[0] += 1
                    if tc_ % cfg.SCALAR_FRAC_DEN < cfg.SCALAR_FRAC_NUM:
                        mbs = msbuf[ci]
                        afull = bq.tile([P, 1024], BF16, tag="A")
                        a = afull[:, :512]
                        nc.scalar.activation(
                            out=a[:], in_=iota_s512[:], func=AF.Abs,
                            bias=mbs[:, slot, 0:1], scale=1.0)
                        nc.scalar.activation(
                            out=B[:], in_=a[:], func=AF.Relu,
                            bias=mbs[:, slot, 2:3],
                            scale=mbs[:, slot, 1:2])
                    else:
                        mb = mbuf[ci]
                        nc.vector.tensor_scalar(
                            out=B[:], in0=iota_v512[:],
                            scalar1=mb[:, slot, 0:1],
                            scalar2=mb[:, slot, 1:2],
                            op0=OP.is_equal, op1=OP.mult)
                    nc.tensor.matmul(
                        out=qpsum[:], lhsT=g[:, slot, :], rhs=B[:],
                        start=first, stop=last)

                ci_of = {}
                for ci, cl in enumerate(calls):
                    ci_of.setdefault((cl[0], cl[1]), []).append(ci)

                func = AF.Relu if layer < cfg.L - 1 else AF.Identity
                for q in range(Q):
                    nw = min(4, W - q * 4)
                    qpsum = eps.tile([P, 512], F32, tag="qp")
                    if l1:
                        tq = st.t1_q[q]
                        cis = ci_of[(q, None)]
                        sizes = [calls[ci][3] for ci in cis]
                        for i in range(tq):
                            rem = i
                            for ci, sz in zip(cis, sizes):
                                if rem < sz:
                                    break
                                rem -= sz
                            emit_tile(ci, rem, qpsum, i == 0, i == tq - 1)
                    else:
                        tq = sum(st.t23_qk[q])
                        done = 0
                        for k in range(CHX):
                            (ci,) = ci_of[(q, k)]
                            for i in range(st.t23_qk[q][k]):
                                emit_tile(ci, i, qpsum,
                                          done == 0, done == tq - 1)
                                done += 1
                    ncol = nw * P
                    nc.scalar.activation(
                        out=h_out[:, q * 512:q * 512 + ncol],
                        in_=qpsum[:, :ncol], func=func,
                        bias=b_cols[:, layer:layer + 1], scale=1.0)

        # ---------------- hw phase ----------------
        def hw_phase(layer, h_in):
            with tc.tile_pool(name=f"hw{layer}", bufs=3) as hp, \
                 tc.tile_pool(name=f"hwps{layer}", bufs=2, space="PSUM") as hps, \
                 tc.tile_pool(name=f"hwps2{layer}", bufs=2, space="PSUM") as hps2:
                nc.sync.dma_start(out=w_sb[:], in_=Ws_d[layer])
                nc.vector.tensor_copy(out=w_bf[:], in_=w_sb[:])
                CWW = 512
                for j0 in range(0, NC, CWW):
                    nj = min(CWW, NC - j0)
                    ps = hps.tile([P, CWW], F32, tag="mm")
                    nc.tensor.matmul(out=ps[:, :nj], lhsT=w_bf[:],
                                     rhs=h_in[:, j0:j0 + nj],
                                     start=True, stop=True)
                    hw_s = hp.tile([P, CWW], F32, tag="hw_s")
                    nc.scalar.activation(out=hw_s[:, :nj], in_=ps[:, :nj],
                                         func=AF.Copy)
                    for q0 in range(0, nj, P):
                        nq = min(P, nj - q0)
                        pt = hps2.tile([P, P], F32, tag="tr")
                        nc.tensor.transpose(out=pt[:nq, :],
                                            in_=hw_s[:, q0:q0 + nq],
                                            identity=ident[:])
                        stg = hp.tile([P, P], BF16, tag="stg")
                        nc.scalar.activation(out=stg[:nq, :], in_=pt[:nq, :],
                                             func=AF.Copy)
                        nc.sync.dma_start(
                            out=hw_shard[j0 + q0:j0 + q0 + nq, :],
                            in_=stg[:nq, :])
            nc.gpsimd.collective_compute(
                "AllGather", OP.bypass, replica_groups=[cores],
                ins=[hw_shard[:, :]], outs=[hw_full[:, :]])

        def dump_h(src_tile):
            dbg_d = nc.declare_dram_parameter("dbg", [P, NCP], F32,
                                              isOutput=True)
            with tc.tile_pool(name="dbg", bufs=2) as dbp:
                CWD = 512
                for s0 in range(0, NCP, CWD):
                    nn = min(CWD, NCP - s0)
                    dt_ = dbp.tile([P, CWD], F32, tag="d")
                    nc.vector.tensor_copy(out=dt_[:, :nn],
                                          in_=src_tile[:, s0:s0 + nn])
                    nc.sync.dma_start(out=dbg_d[:, s0:s0 + nn],
                                      in_=dt_[:, :nn])

        with nc.named_scope("layer1"):
            edge_pass(0, hT_a)
        if DEBUG_STAGE == 1:
            dump_h(hT_a)
        with nc.named_scope("hw2"):
            hw_phase(1, hT_a)
        with nc.named_scope("layer2"):
            edge_pass(1, hT_b)
        if DEBUG_STAGE == 2:
            dump_h(hT_b)
        with nc.named_scope("hw3"):
            hw_phase(2, hT_b)
        with nc.named_scope("layer3"):
            edge_pass(2, hT_a)
        if DEBUG_STAGE == 3:
            dump_h(hT_a)

        # ---------------- pooling ----------------
        with nc.named_scope("pool"), \
             tc.tile_pool(name="po", bufs=3) as po, \
             tc.tile_pool(name="po_ps", bufs=2, space="PSUM") as po_ps, \
             tc.tile_pool(name="po_acc", bufs=1, space="PSUM") as po_acc:
            pm = po.tile([P, W], F32, tag="pm")
            nc.sync.dma_start(out=pm[:], in_=pool_meta[:, :])
            gcols = po.tile([P, 2], I32, tag="gcols")
            nc.sync.dma_start(out=gcols[:], in_=gid_cols[:, :])
            recip_sb = po.tile([P, GW], F32, tag="recip")
            nc.sync.dma_start(out=recip_sb[:], in_=recip_pm[:, :])

            acc = po_acc.tile([P, GS], F32)
            for t in range(W):
                pt = po_ps.tile([P, P], BF16, tag="ptr")
                nc.tensor.transpose(out=pt[:], in_=hT_a[:, t * P:(t + 1) * P],
                                    identity=ident_bf[:])
                h3nm = po.tile([P, P], BF16, tag="h3nm")
                nc.scalar.activation(out=h3nm[:], in_=pt[:], func=AF.Copy)
                Bp = po.tile([P, GS], BF16, tag="Bp")
                nc.vector.tensor_scalar(
                    out=Bp[:], in0=iota_bf[:],
                    scalar1=pm[:, t:t + 1], scalar2=None,
                    op0=OP.is_equal)
                nc.tensor.matmul(out=acc[:], lhsT=h3nm[:], rhs=Bp[:],
                                 start=(t == 0), stop=(t == W - 1))

            def dummy_gather():
                dz = po.tile([P, 1, P], BF16, tag="dz")
                zi = po.tile([P, 8], I16, tag="zi")
                nc.vector.memset(zi[:], 0)
                gi = nc.gpsimd.dma_gather(
                    out_ap=dz[:], in_ap=t1_dram[:, :], idxs_ap=zi[:],
                    num_idxs=P, num_idxs_reg=P, elem_size=H,
                    single_packet=False, queue_num=pd["i"] % NQ)
                chain_pool_dma(gi)

            zt = po.tile([P, P], F32, tag="zt")
            nc.vector.memset(zt[:], 0.0)
            for r0 in range(0, cfg.G + GS, P):
                nc.sync.dma_start(out=pooled_nm[r0:r0 + P, :], in_=zt[:])

            acc_sb = po.tile([P, GS], F32, tag="acc_sb")
            nc.scalar.activation(out=acc_sb[:], in_=acc[:], func=AF.Copy)
            for half in range(2):
                pt = po_ps.tile([P, P], F32, tag="ptr2")
                nc.tensor.transpose(out=pt[:],
                                    in_=acc_sb[:, half * P:(half + 1) * P],
                                    identity=ident[:])
                rows = po.tile([P, P], F32, tag="rows")
                nc.scalar.activation(out=rows[:], in_=pt[:], func=AF.Copy)
                while pd["i"] % NQ != 0:
                    dummy_gather()  # scatters run on queue 0: align lane
                si = nc.gpsimd.indirect_dma_start(
                    out=pooled_nm[:, :],
                    out_offset=IndirectOffsetOnAxis(
                        ap=gcols[:, half:half + 1], axis=0),
                    in_=rows[:], in_offset=None)
                chain_pool_dma(si)

            nc.gpsimd.collective_compute(
                "AllReduce", OP.add, replica_groups=[cores],
                ins=[pooled_nm[:, :]], outs=[pooled_sum[:, :]])

            for gw in range(GW):
                ot = po.tile([P, H], F32, tag="ot")
                nc.sync.dma_start(out=ot[:],
                                  in_=pooled_sum[gw * P:(gw + 1) * P, :])
                os = po.tile([P, H], F32, tag="os")
                nc.vector.tensor_scalar(
                    out=os[:], in0=ot[:], scalar1=recip_sb[:, gw:gw + 1],
                    scalar2=None, op0=OP.mult)
                nc.sync.dma_start(out=out_d[gw * P:(gw + 1) * P, :],
                                  in_=os[:])

    return nc


# --------------------------------------------------------------------------
# entry point: full inputs -> full output
# --------------------------------------------------------------------------

_CACHE = {}


def _get_compiled(cfg, st_key, st):
    if st_key not in _CACHE:
        nc = build_nc(cfg, st)
        nc.finalize()
        _CACHE[st_key] = nc
    return _CACHE[st_key]


def kernel(x, edge_index, batch, emb_table, Ws, bs):
    cfg = Cfg()  # full problem size, hardcoded
    st, in_maps = preprocess(x, edge_index, batch, emb_table, Ws, bs, cfg)
    st_key = (tuple(st.t1_q), tuple(tuple(r) for r in st.t23_qk))
    nc = _get_compiled(cfg, st_key, st)

    from concourse.bass_utils import run_bass_kernel_spmd

    res = run_bass_kernel_spmd(nc, in_maps, list(range(cfg.C)))
    return np.ascontiguousarray(res.results[0]["out"])
